# revision 6
# baseline (speedup 1.0000x reference)
"""Trainium2 Bass kernel for pairwise Tang-Toennies dispersion energy.

Problem: for P=3.2M random atom pairs over N=100k atoms in a periodic box,
    ene[p] = -(c6[ti,tj] * f6(b[ti,tj]*r)) / r^6   if r <= cutoff else 0
with r the minimum-image distance and f6 the 6th-order Tang-Toennies damper.

Only ~2% of random pairs fall inside the 10A cutoff.  The kernel is a
three-stage device funnel; the host only quantizes coordinates (an O(N)
table prep), gathers per-pair planes (pure index routing), and compacts by
device-produced integer verdicts:

  A1 (dense, 3.2M slots):  u8-quantized z-axis screen.  Coordinates are
      quantized host-side to q = round(256*x/L) mod 256, so the device's
      d = jz - iz followed by a minimum-image fold is exact integer
      arithmetic in f32.  Each tile outputs one byte per pair encoding the
      folded z-distance; a pair can only have r <= cutoff if
      m_z <= floor(256*cutoff/L + 1) = 43 (the +1 covers quantization).
      One tile uses the single-op add_range_wrap fold (DVE), the others
      the two-op |.| / |.|-128 fold (Act) -- chosen to balance engines.
      ~34% of pairs survive.
  A2 (survivors of A1): full 3-axis r^2 in q-space, against
      (256*cutoff/L + sqrt(3))^2 = 1971 -- again conservative by the
      quantization error, so no true pair is lost.  ~2.2% of all pairs
      survive.
  B  (sparse, ~70k slots): exact f32 energy, identical math to the
      reference: min-image via add_range_wrap (bit-identical to the
      round() form), r/r^-6 via Ln/Exp, Tang-Toennies f6 via an
      Estrin-form polynomial, exact series fallback for r^2<=0.4 where
      the 1 - e^-u*poly form is pure f32 cancellation noise.  B applies
      the exact f32 cutoff, so the final mask does not depend on the
      screen margins.  All Act functions (Square/Ln/Exp) live in one
      activation-table set; a post-compile fixup pins the compiler's
      greedy per-function table loads to that set so the hot loop has no
      ~1.3us table switches.

Work is sharded by slots: every core gets the same slot count for all
kernels, so the 8 cores are perfectly balanced.
"""

import contextlib

import numpy as np

import concourse.bacc as bacc
import concourse.bass as bass
import concourse.mybir as mybir
from concourse.tile import TileContext
from concourse.bass_utils import run_bass_kernel_spmd

F32 = mybir.dt.float32
U8 = mybir.dt.uint8
I8 = mybir.dt.int8
AF = mybir.ActivationFunctionType
OP = mybir.AluOpType

N_CORES = 8
Q = 256  # quantization bins per box length

# kernel A1 (z screen): slots/core = 128 * K1 * T1
K1 = 1042
T1 = 3
SLOTS_A1 = 128 * K1 * T1         # 400,128 per core (3,201,024 total)

# kernel A2 (3-axis screen): slots/core = 128 * K2 * T2
K2 = 568
T2 = 2
SLOTS_A2 = 128 * K2 * T2         # 145,408 per core (1,163,264 total)

# kernel B (exact energy): slots/core = 128 * KB * TB
KB = 40
TB = 2
SLOTS_B = 128 * KB * TB          # 10,240 per core (81,920 total)


def _register_consts(nc, values):
    """Pre-register [128,1] SBUF constants so float biases work on Act."""
    for v in values:
        t = nc.alloc_sbuf_tensor(f"const-f32-{v}", [128, 1], F32)
        nc.gpsimd.memset(t.ap(), v)
        nc.const_aps.aps[(F32, v)] = t.ap()
    nc.all_engine_barrier()


def _zthr(cutoff, L):
    # pass if folded z-distance (integer, quantization err <= 1) <= zmax
    return int(np.floor(cutoff / L * Q + 1.0))


def _r2thr(cutoff, L):
    # pass if q-space r^2 (integer, per-axis err <= 1) <= r2max
    return int(np.floor((cutoff / L * Q + np.sqrt(3.0)) ** 2))


def build_a1(cutoff, L, reps=1):
    """z-axis screen: packed [jz | iz] u8 rows -> one verdict byte/slot.

    Tile 0 writes zw = add_range_wrap(jz - iz) as int8 (pass: |zw| <= 43);
    tiles 1..T1-1 write zf = ||jz - iz| - 128| as uint8 (pass: zf >= 85).
    Split keeps DVE and Act equally busy.
    """
    nc = bacc.Bacc(trn_type="TRN2", target_bir_lowering=False)
    _register_consts(nc, [-128.0])
    zz = nc.dram_tensor("zz", [T1 * 128 * 2 * K1], U8, kind="ExternalInput")
    zw_d = nc.dram_tensor("zw", [128 * K1], I8, kind="ExternalOutput")
    zf_d = nc.dram_tensor("zf", [(T1 - 1) * 128 * K1], U8,
                          kind="ExternalOutput")

    with TileContext(nc) as tc:
        rep_ctx = tc.For_i(0, reps, 1) if reps > 1 else contextlib.nullcontext()
        with tc.tile_pool(name="io", bufs=3) as pio, \
             tc.tile_pool(name="work", bufs=3) as pw, rep_ctx:
            # all input DMAs first (SP queue): nothing ahead of them waits
            # on compute, so the loads of every tile stream back-to-back.
            ws = []
            for t in range(T1):
                w = pio.tile([128, 2 * K1], U8, tag=f"w{t}", name=f"w{t}")
                nc.sync.dma_start(
                    out=w[:],
                    in_=zz[bass.ts(t, 128 * 2 * K1)].rearrange(
                        "(p x) -> p x", x=2 * K1))
                ws.append(w)
            outs = []
            for t in range(T1):
                w = ws[t]
                d = pw.tile([128, K1], F32, tag=f"d{t}", name=f"d{t}")
                nc.vector.tensor_tensor(out=d[:], in0=w[:, 0:K1],
                                        in1=w[:, K1:2 * K1], op=OP.subtract)
                if t == 0:
                    zw = pio.tile([128, K1], I8, tag="zw")
                    nc.vector.add_range_wrap(out=zw[:], in_=d[:], shift=0.0,
                                             bound=128.0, period=256.0)
                    outs.append((zw, zw_d[:]))
                else:
                    e = pw.tile([128, K1], F32, tag=f"e{t}", name=f"e{t}")
                    nc.scalar.activation(e[:], d[:], AF.Abs)
                    zf = pio.tile([128, K1], U8, tag=f"zf{t}")
                    nc.scalar.activation(zf[:], e[:], AF.Abs, bias=-128.0)
                    outs.append((zf, zf_d[bass.ts(t - 1, 128 * K1)]))
            # output DMAs on the Activation HWDGE queue: their compute
            # waits stall only the (already finished) Act stream, never
            # the SP input queue.
            for tile, dst in outs:
                nc.scalar.dma_start(
                    out=dst.rearrange("(p x) -> p x", x=K1), in_=tile[:])
    nc.compile()
    return nc


def build_a2(cutoff, L, reps=1):
    """3-axis sphere screen in q-space on A1 survivors.

    Input chunks per tile row: [jx ix jy iy jz iz] (u8).  mask=1 iff
    sum of folded-axis squares <= 1971.  Tile 0 folds via add_range_wrap
    (DVE-heavy), tile 1 via the double-abs chain (Act-heavy).
    """
    r2max = float(_r2thr(cutoff, L))
    nc = bacc.Bacc(trn_type="TRN2", target_bir_lowering=False)
    _register_consts(nc, [-128.0, 128.0, r2max + 0.5])
    pk = nc.dram_tensor("pk", [T2 * 128 * 6 * K2], U8, kind="ExternalInput")
    mask_d = nc.dram_tensor("mask", [SLOTS_A2 // T2 * T2], U8,
                            kind="ExternalOutput")
    K = K2

    with TileContext(nc) as tc:
        rep_ctx = tc.For_i(0, reps, 1) if reps > 1 else contextlib.nullcontext()
        with tc.tile_pool(name="io", bufs=3) as pio, \
             tc.tile_pool(name="work", bufs=3) as pw, rep_ctx:
            ws = []
            for t in range(T2):
                w = pio.tile([128, 6 * K], U8, tag=f"w{t}", name=f"w{t}")
                nc.sync.dma_start(
                    out=w[:],
                    in_=pk[bass.ts(t, 128 * 6 * K)].rearrange(
                        "(p x) -> p x", x=6 * K))
                ws.append(w)
            outs = []
            for t in range(T2):
                w = ws[t]
                jv = bass.AP(w.tensor, 0, [w[:].ap[0], [2 * K, 3], [1, K]])
                iv = bass.AP(w.tensor, K, [w[:].ap[0], [2 * K, 3], [1, K]])
                d = pw.tile([128, 3 * K], F32, tag=f"d{t}", name=f"d{t}")
                d3 = d[:].rearrange("p (a k) -> p a k", a=3)
                nc.vector.tensor_tensor(out=d3, in0=jv, in1=iv, op=OP.subtract)
                sq = pw.tile([128, 3 * K], F32, tag=f"sq{t}", name=f"sq{t}")
                if t == 0:
                    dw = pw.tile([128, 3 * K], F32, tag="dw", name="dw")
                    nc.vector.add_range_wrap(out=dw[:], in_=d[:], shift=0.0,
                                             bound=128.0, period=256.0)
                    nc.scalar.activation(sq[:], dw[:], AF.Square)
                else:
                    e = pw.tile([128, 3 * K], F32, tag=f"e{t}", name=f"e{t}")
                    nc.scalar.activation(e[:], d[:], AF.Abs)
                    f = pw.tile([128, 3 * K], F32, tag=f"f{t}", name=f"f{t}")
                    nc.scalar.activation(f[:], e[:], AF.Abs, bias=-128.0)
                    # (128 - f)^2 = Square(-f + 128)
                    nc.scalar.activation(sq[:], f[:], AF.Square, scale=-1.0,
                                         bias=128.0)
                sq3 = sq[:].rearrange("p (a k) -> p a k", a=3)
                r2 = pw.tile([128, K], F32, tag=f"r2{t}", name=f"r2{t}")
                if t == 0:
                    nc.gpsimd.tensor_tensor(out=r2[:], in0=sq3[:, 0, :],
                                            in1=sq3[:, 1, :], op=OP.add)
                    nc.vector.tensor_tensor(out=r2[:], in0=r2[:],
                                            in1=sq3[:, 2, :], op=OP.add)
                else:
                    nc.vector.tensor_tensor(out=r2[:], in0=sq3[:, 0, :],
                                            in1=sq3[:, 1, :], op=OP.add)
                    nc.gpsimd.tensor_tensor(out=r2[:], in0=r2[:],
                                            in1=sq3[:, 2, :], op=OP.add)
                mk = pio.tile([128, K], U8, tag=f"mk{t}")
                if t == 0:
                    # sign(-r2 + (r2max+.5)) -> u8: 1 iff r2 <= r2max
                    nc.scalar.activation(mk[:], r2[:], AF.Sign, scale=-1.0,
                                         bias=r2max + 0.5)
                else:
                    nc.vector.tensor_scalar(out=mk[:], in0=r2[:],
                                            scalar1=r2max + 0.5,
                                            scalar2=None, op0=OP.is_le)
                outs.append((mk, mask_d[bass.ts(t, 128 * K)]))
            for tile, dst in outs:
                nc.scalar.dma_start(
                    out=dst.rearrange("(p x) -> p x", x=K), in_=tile[:])
    nc.compile()
    return nc


def _patch_act_tables(nc, set_id=6):
    """Pin every compiler-inserted activation-table load to one set and
    drop the now-redundant reloads.

    The insertion pass picks the first table set containing each function
    (exp -> set 0, ln -> set 5), which costs two ~1.3us reloads per loop
    iteration in kernel B.  Set 6 (natural_log_exp_and_others) holds
    ln+exp+square together, so a single load suffices.  The inserted
    loads carry no semaphore activity (sync None), so dropping all but
    the first is safe.
    """
    loads = []
    for blk in nc.m.functions[0].blocks:
        for inst in blk.instructions:
            if isinstance(inst, mybir.InstLoadActFuncSet):
                assert inst.sync_info is None
                loads.append((blk, inst))
    for i, (blk, inst) in enumerate(loads):
        if i == 0:
            inst.act_func_set_id = set_id
        else:
            blk.instructions.remove(inst)
    return nc


def build_b(Ls, cutoff, reps=1):
    """Sparse exact energy on compacted in-cutoff slots.

    Input chunks per tile row: [jx ix jy iy jz iz cp bp] (f32, normalized
    coords, cp = -c6[ti,tj], bp = b[ti,tj]).
    """
    nc = bacc.Bacc(trn_type="TRN2", target_bir_lowering=False)
    pk_d = nc.dram_tensor("pk", [TB * 128 * 8 * KB], F32, kind="ExternalInput")
    ene_d = nc.dram_tensor("ene", [SLOTS_B], F32, kind="ExternalOutput")
    c2 = float(np.float32(cutoff) ** 2)
    K = KB
    L = float(Ls[0])

    with TileContext(nc) as tc:
        rep_ctx = tc.For_i(0, reps, 1) if reps > 1 else contextlib.nullcontext()
        with tc.tile_pool(name="io", bufs=3) as pio, \
             tc.tile_pool(name="work", bufs=3) as pw, rep_ctx:
            ws = []
            for t in range(TB):
                w = pio.tile([128, 8 * K], F32, tag=f"w{t}", name=f"w{t}")
                nc.sync.dma_start(
                    out=w[:],
                    in_=pk_d[bass.ts(t, 128 * 8 * K)].rearrange(
                        "(p x) -> p x", x=8 * K))
                ws.append(w)
            fouts = []
            for t in range(TB):
                w = ws[t]
                wv = w[:].rearrange("p (c k) -> p c k", c=8)
                cp = wv[:, 6, :]
                bp = wv[:, 7, :]

                # ---- geometry: min-image wrap (== reference's round form
                # for |d| < 1), then r^2 = sum (L*dw)^2 ----
                jv = bass.AP(w.tensor, 0, [w[:].ap[0], [2 * K, 3], [1, K]])
                iv = bass.AP(w.tensor, K, [w[:].ap[0], [2 * K, 3], [1, K]])
                d = pw.tile([128, 3 * K], F32, tag="d", name="d")
                d3 = d[:].rearrange("p (a k) -> p a k", a=3)
                nc.vector.tensor_tensor(out=d3, in0=jv, in1=iv,
                                        op=OP.subtract)
                dw = pw.tile([128, 3 * K], F32, tag="dw", name="dw")
                nc.vector.add_range_wrap(out=dw[:], in_=d[:], shift=0.0,
                                         bound=0.5, period=1.0)
                sq = pw.tile([128, 3 * K], F32, tag="sq", name="sq")
                nc.scalar.activation(sq[:], dw[:], AF.Square, scale=L)
                sq3 = sq[:].rearrange("p (a k) -> p a k", a=3)
                r2 = pw.tile([128, K], F32, tag="r2", name="r2")
                nc.gpsimd.tensor_tensor(out=r2[:], in0=sq3[:, 0, :],
                                        in1=sq3[:, 1, :], op=OP.add)
                nc.gpsimd.tensor_tensor(out=r2[:], in0=r2[:],
                                        in1=sq3[:, 2, :], op=OP.add)

                # ---- r, r^-6 via one Ln + two Exp ----
                lr2 = pw.tile([128, K], F32, tag="lr2")
                nc.scalar.activation(lr2[:], r2[:], AF.Ln)
                rr = pw.tile([128, K], F32, tag="rr")
                nc.scalar.activation(rr[:], lr2[:], AF.Exp, scale=0.5)
                ir6 = pw.tile([128, K], F32, tag="ir6")
                nc.scalar.activation(ir6[:], lr2[:], AF.Exp, scale=-3.0)

                u = pw.tile([128, K], F32, tag="u")
                nc.vector.tensor_tensor(out=u[:], in0=bp, in1=rr[:],
                                        op=OP.mult)
                em = pw.tile([128, K], F32, tag="em")
                nc.scalar.activation(em[:], u[:], AF.Exp, scale=-1.0)
                wq = pw.tile([128, K], F32, tag="wq")
                nc.scalar.activation(wq[:], u[:], AF.Square)

                # ---- poly(u) = sum_0^6 u^k/k!, balanced Estrin:
                # p = (1+u + w*(1/2+u/6)) + w^2*((1/24 + u/120) + w/720)
                a_ = pw.tile([128, K], F32, tag="a_")
                nc.vector.tensor_scalar(out=a_[:], in0=u[:], scalar1=1.0 / 6.0,
                                        scalar2=0.5, op0=OP.mult, op1=OP.add)
                bq = pw.tile([128, K], F32, tag="bq")
                nc.gpsimd.tensor_scalar(out=bq[:], in0=u[:], scalar1=1.0 / 120.0,
                                        scalar2=1.0 / 24.0, op0=OP.mult,
                                        op1=OP.add)
                t1 = pw.tile([128, K], F32, tag="t1")
                nc.gpsimd.tensor_scalar(out=t1[:], in0=u[:], scalar1=1.0,
                                        scalar2=None, op0=OP.add)
                w2 = pw.tile([128, K], F32, tag="w2")
                nc.scalar.activation(w2[:], wq[:], AF.Square)
                d2 = pw.tile([128, K], F32, tag="d2")
                nc.vector.scalar_tensor_tensor(out=d2[:], in0=wq[:],
                                               scalar=1.0 / 720.0, in1=bq[:],
                                               op0=OP.mult, op1=OP.add)
                g = pw.tile([128, K], F32, tag="g")
                nc.vector.tensor_tensor(out=g[:], in0=wq[:], in1=a_[:],
                                        op=OP.mult)
                nc.vector.tensor_tensor(out=g[:], in0=g[:], in1=t1[:],
                                        op=OP.add)
                h = pw.tile([128, K], F32, tag="h")
                nc.vector.tensor_tensor(out=h[:], in0=w2[:], in1=d2[:],
                                        op=OP.mult)
                p = pw.tile([128, K], F32, tag="p")
                nc.vector.tensor_tensor(out=p[:], in0=g[:], in1=h[:],
                                        op=OP.add)

                # ---- ene = B'*poly - A' with A' = c6/r^6.  cp = -c6, so
                # An = -A', Bn = An*em = -B', ene = (-1*Bn)*poly + An. ----
                An = pw.tile([128, K], F32, tag="An")
                nc.vector.tensor_tensor(out=An[:], in0=cp, in1=ir6[:],
                                        op=OP.mult)
                Bn = pw.tile([128, K], F32, tag="Bn")
                nc.vector.tensor_tensor(out=Bn[:], in0=An[:], in1=em[:],
                                        op=OP.mult)
                ene = pw.tile([128, K], F32, tag="ene")
                nc.vector.scalar_tensor_tensor(out=ene[:], in0=Bn[:],
                                               scalar=-1.0, in1=p[:],
                                               op0=OP.mult, op1=OP.mult)
                nc.vector.tensor_tensor(out=ene[:], in0=ene[:], in1=An[:],
                                        op=OP.add)

                # ---- small-u exact series: f6 = em*u^7/5040*(1+u/8+u^2/72)
                # (the direct 1-em*poly form is f32 cancellation noise there;
                # es = (Bn/5040)*u7*S = -B'/5040*u7*S, correctly negative)
                u3 = pw.tile([128, K], F32, tag="u3")
                nc.gpsimd.tensor_tensor(out=u3[:], in0=u[:], in1=wq[:],
                                        op=OP.mult)
                u6 = pw.tile([128, K], F32, tag="u6")
                nc.scalar.activation(u6[:], u3[:], AF.Square)
                u7 = pw.tile([128, K], F32, tag="u7")
                nc.gpsimd.tensor_tensor(out=u7[:], in0=u6[:], in1=u[:],
                                        op=OP.mult)
                s1u = pw.tile([128, K], F32, tag="s1u")
                nc.gpsimd.tensor_scalar(out=s1u[:], in0=u[:], scalar1=1.0 / 8.0,
                                        scalar2=1.0, op0=OP.mult, op1=OP.add)
                S = pw.tile([128, K], F32, tag="S")
                nc.vector.scalar_tensor_tensor(out=S[:], in0=wq[:],
                                               scalar=1.0 / 72.0, in1=s1u[:],
                                               op0=OP.mult, op1=OP.add)
                es = pw.tile([128, K], F32, tag="es")
                nc.gpsimd.tensor_tensor(out=es[:], in0=u7[:], in1=S[:],
                                        op=OP.mult)
                nc.vector.scalar_tensor_tensor(out=es[:], in0=Bn[:],
                                               scalar=1.0 / 5040.0, in1=es[:],
                                               op0=OP.mult, op1=OP.mult)
                mu = pw.tile([128, K], U8, tag="mu")
                nc.gpsimd.tensor_scalar(out=mu[:], in0=r2[:], scalar1=0.4,
                                        scalar2=None, op0=OP.is_le)
                nc.vector.select(out=ene[:], mask=mu[:], on_true=es[:],
                                 on_false=ene[:])

                # ---- exact f32 cutoff ----
                eout = pio.tile([128, K], F32, tag=f"eout{t}")
                nc.vector.scalar_tensor_tensor(out=eout[:], in0=r2[:],
                                               scalar=c2, in1=ene[:],
                                               op0=OP.is_le, op1=OP.mult)
                fouts.append((eout, ene_d[bass.ts(t, 128 * K)]))
            for tile, dst in fouts:
                nc.scalar.dma_start(
                    out=dst.rearrange("(p x) -> p x", x=K), in_=tile[:])
    nc.compile()
    _patch_act_tables(nc, set_id=6)
    return nc


_NC_CACHE = {}


def _get_nc(builder, key, *args, **kw):
    if key not in _NC_CACHE:
        _NC_CACHE[key] = builder(*args, **kw)
    return _NC_CACHE[key]


def _host_reference(coords, pairs, box, c6, b, cutoff, atom_types):
    # numpy fallback for non-cubic boxes (not hit by the real inputs)
    dr = coords[pairs[:, 1]] - coords[pairs[:, 0]]
    inv_box = np.linalg.inv(box)
    dr = dr - np.round(dr @ inv_box) @ box
    r = np.sqrt((dr * dr).sum(1))
    ti = atom_types[pairs[:, 0]]
    tj = atom_types[pairs[:, 1]]
    u = b[ti, tj] * r
    poly = 1.0 + u * (1.0 + u / 2.0 * (1.0 + u / 3.0 * (1.0 + u / 4.0 *
                     (1.0 + u / 5.0 * (1.0 + u / 6.0)))))
    f6 = 1.0 - np.exp(-u) * poly
    ene = -(c6[ti, tj] * f6) / r ** 6
    return np.where(r <= cutoff, ene, 0.0).astype(np.float32)


def _bufs_a1(qz, pi, pj):
    """Per-core A1 input: packed [T1, 128, [jz | iz]] u8 rows."""
    P = pi.shape[0]
    total = N_CORES * SLOTS_A1
    jz = np.zeros(total, np.uint8)
    iz = np.full(total, 128, np.uint8)
    jz[:P] = qz[pj]
    iz[:P] = qz[pi]
    pk = np.stack([jz.reshape(N_CORES, T1, 128, K1),
                   iz.reshape(N_CORES, T1, 128, K1)], axis=3)
    return np.ascontiguousarray(pk).reshape(N_CORES, -1)


def _decode_a1(zw, zf, zmax):
    """Per-core A1 verdicts -> boolean pass mask over SLOTS_A1 slots."""
    ok_w = np.abs(zw.view(np.int8).astype(np.int16)) <= zmax
    ok_f = zf >= (128 - zmax)
    return np.concatenate([ok_w, ok_f])


def _bufs_a2(q, pi, pj, sl):
    """Per-core A2 input for survivor indices sl: [T2,128,[6 chunks]] u8."""
    cap = N_CORES * SLOTS_A2
    n = sl.shape[0]
    pk = np.empty((6, cap), np.uint8)
    for ci in range(3):
        pk[2 * ci, n:] = 0
        pk[2 * ci, :n] = q[pj[sl], ci]
        pk[2 * ci + 1, n:] = 128
        pk[2 * ci + 1, :n] = q[pi[sl], ci]
    pk = pk.reshape(6, N_CORES, T2, 128, K2).transpose(1, 2, 3, 0, 4)
    return np.ascontiguousarray(pk).reshape(N_CORES, -1)


def _bufs_b(coords_n, c6, b, pi, pj, ti, tj, sl):
    """Kernel B packed input (chunks [jx,ix,jy,iy,jz,iz,cp,bp]) for one
    chunk of survivor indices sl.  cp = -c6."""
    cap = N_CORES * SLOTS_B
    n = sl.shape[0]
    pk = np.empty((8, cap), np.float32)
    for ci in range(3):
        pk[2 * ci, n:] = 0.25
        pk[2 * ci, :n] = coords_n[pj[sl], ci]
        pk[2 * ci + 1, n:] = 0.0
        pk[2 * ci + 1, :n] = coords_n[pi[sl], ci]
    pk[6, n:] = 0.0
    pk[6, :n] = -c6[ti, tj]
    pk[7, n:] = 1.0
    pk[7, :n] = b[ti, tj]
    pk = pk.reshape(8, N_CORES, TB, 128, KB).transpose(1, 2, 3, 0, 4)
    return np.ascontiguousarray(pk).reshape(N_CORES, -1)


def kernel(coords, pairs, box, c6, b, cutoff, atom_types):
    coords = np.asarray(coords, np.float32)
    pairs = np.asarray(pairs)
    box = np.asarray(box, np.float32)
    c6 = np.asarray(c6, np.float32)
    b = np.asarray(b, np.float32)
    atom_types = np.asarray(atom_types).astype(np.int64)
    cutoff = float(np.asarray(cutoff))

    offdiag = box - np.diag(np.diag(box))
    Ls = np.diag(box)
    if (np.any(offdiag != 0.0) or Ls[0] != Ls[1] or Ls[0] != Ls[2]
            or pairs.shape[0] > N_CORES * SLOTS_A1
            or cutoff / Ls[0] * Q + np.sqrt(3.0) >= 127):
        return _host_reference(coords, pairs, box, c6, b, cutoff, atom_types)
    L = float(Ls[0])

    P = pairs.shape[0]
    pi = np.ascontiguousarray(pairs[:, 0]).astype(np.int64)
    pj = np.ascontiguousarray(pairs[:, 1]).astype(np.int64)
    coords_n = coords / np.float32(L)
    q = np.mod(np.rint(coords_n * Q), Q).astype(np.uint8)
    zmax = _zthr(cutoff, L)

    # ---- A1: z-axis screen over all pairs ----
    nc_a1 = _get_nc(build_a1, ("a1", round(cutoff, 6), L), cutoff, L)
    in_a1 = _bufs_a1(q[:, 2], pi, pj)
    res1 = run_bass_kernel_spmd(nc_a1, [{"zz": in_a1[c]}
                                        for c in range(N_CORES)],
                                core_ids=list(range(N_CORES)))
    ok1 = np.concatenate([_decode_a1(res1.results[c]["zw"],
                                     res1.results[c]["zf"], zmax)
                          for c in range(N_CORES)])
    zidx = np.flatnonzero(ok1[:P])

    # ---- A2: full sphere screen in q-space (chunked if ever needed) ----
    nc_a2 = _get_nc(build_a2, ("a2", round(cutoff, 6), L), cutoff, L)
    cap2 = N_CORES * SLOTS_A2
    surv = []
    for lo in range(0, max(zidx.shape[0], 1), cap2):
        sl = zidx[lo:lo + cap2]
        in_a2 = _bufs_a2(q, pi, pj, sl)
        res2 = run_bass_kernel_spmd(nc_a2, [{"pk": in_a2[c]}
                                            for c in range(N_CORES)],
                                    core_ids=list(range(N_CORES)))
        m2 = np.concatenate([res2.results[c]["mask"]
                             for c in range(N_CORES)])
        surv.append(sl[np.flatnonzero(m2[:sl.shape[0]])])
    idx = np.concatenate(surv) if surv else np.empty(0, np.int64)

    # ---- B: exact energies for survivors (chunked if ever needed) ----
    nc_b = _get_nc(build_b, ("b", round(cutoff, 6), L), (L, L, L), cutoff)
    capb = N_CORES * SLOTS_B
    ene_s = np.empty(idx.shape[0], np.float32)
    for lo in range(0, max(idx.shape[0], 1), capb):
        sl = idx[lo:lo + capb]
        ti = atom_types[pi[sl]]
        tj = atom_types[pj[sl]]
        pkb = _bufs_b(coords_n, c6, b, pi, pj, ti, tj, sl)
        res_b = run_bass_kernel_spmd(nc_b, [{"pk": pkb[c]}
                                            for c in range(N_CORES)],
                                     core_ids=list(range(N_CORES)))
        ene_full = np.concatenate([res_b.results[c]["ene"]
                                   for c in range(N_CORES)])
        ene_s[lo:lo + sl.shape[0]] = ene_full[:sl.shape[0]]

    out = np.zeros(P, np.float32)
    out[idx] = ene_s
    return out
